# revision 1
# baseline (speedup 1.0000x reference)
"""DGCNN regression bass kernel for TRN2 — one batch element per NeuronCore.

Pipeline per edge-conv block (C -> O):
  scores S[n,m] = x_n.x_m - xx[m]/2  (ranking-equivalent to reference pd)
  per row-tile (128 points): 16 chunkwise max8 + max_index -> 128 merged
  candidates; 3 merged match_replace rounds with kill-diff index decode ->
  exact top-20 neighbor indices; gpsimd ap_gather on u = Wn@x + reduce-max
  over k; M + v (v = (Wc-Wn)@x) -> BN+LeakyReLU via ACT Prelu.
Final: z = lrelu(bn5(W5 @ [x1;x2;x3;x4])), out = wreg @ z.
"""
import os
import numpy as np
import concourse.bass as bass
import concourse.mybir as mybir
from concourse import bacc, tile
from concourse.masks import make_identity

F32 = mybir.dt.float32
U16 = mybir.dt.uint16
I16 = mybir.dt.int16
AOT = mybir.AluOpType
ACT_F = mybir.ActivationFunctionType
NEG = -3.0e38
N = 4096
K = 20
NTILES = N // 128
EPS = 1e-5


def build(num_devices=8, reps=1):
    nc = bacc.Bacc("TRN2", target_bir_lowering=False, debug=False,
                   num_devices=num_devices)
    x_d = nc.dram_tensor("x", [1, N], F32, kind="ExternalInput").ap()
    w_d = {}
    for name, shape in [("w1", (64, 2)), ("w2", (64, 128)), ("w3", (128, 128)),
                        ("w4", (256, 256)), ("w5", (1024, 512)), ("wreg", (1, 1024))]:
        w_d[name] = nc.dram_tensor(name, list(shape), F32, kind="ExternalInput").ap()
    bn_d = {}
    for tag, c in [("1", 64), ("2", 64), ("3", 128), ("4", 256), ("5", 1024)]:
        for p in ("g", "b", "rm", "rv"):
            bn_d[p + tag] = nc.dram_tensor(p + tag, [1, c], F32, kind="ExternalInput").ap()
    out_d = nc.dram_tensor("out", [1, N], F32, kind="ExternalOutput").ap()

    with tile.TileContext(nc) as tc:
        for _ in range(reps):
            _build_body(nc, tc, x_d, w_d, bn_d, out_d)
    nc.compile()
    return nc


def _bn_consts(nc, pool, bn_d, tag, O):
    """Load g/b/rm/rv; produce scale/bias as [128, ceil(O/128)] (col h = chans h*128..)."""
    H = (O + 127) // 128
    P = min(O, 128)
    g = pool.tile([P, H], F32, name=f"g{tag}")
    b = pool.tile([P, H], F32, name=f"b{tag}")
    rm = pool.tile([P, H], F32, name=f"rm{tag}")
    sc = pool.tile([P, H], F32, name=f"sc{tag}")
    bi = pool.tile([P, H], F32, name=f"bi{tag}")
    nc.sync.dma_start(out=g, in_=bn_d["g" + tag].rearrange("1 (h p) -> p h", p=P))
    nc.sync.dma_start(out=b, in_=bn_d["b" + tag].rearrange("1 (h p) -> p h", p=P))
    nc.sync.dma_start(out=rm, in_=bn_d["rm" + tag].rearrange("1 (h p) -> p h", p=P))
    nc.sync.dma_start(out=sc, in_=bn_d["rv" + tag].rearrange("1 (h p) -> p h", p=P))
    nc.vector.tensor_scalar_add(sc, sc, EPS)
    nc.scalar.sqrt(sc, sc)
    nc.vector.reciprocal(sc, sc)
    nc.vector.tensor_tensor(out=sc, in0=sc, in1=g, op=AOT.mult)
    nc.vector.tensor_tensor(out=bi, in0=rm, in1=sc, op=AOT.mult)
    nc.vector.tensor_tensor(out=bi, in0=b, in1=bi, op=AOT.subtract)
    return sc, bi


def _transpose_to(nc, tc, pp, pool, src_ap, dst_ap, ident):
    """dst[j, i] = src[i, j] via PE transpose in 128x128 blocks (fp32).
    src_ap: [R, Cc] SBUF; dst_ap: [Cc, R] SBUF. R, Cc multiples of / up to 128."""
    R, Cc = src_ap.shape
    for i0 in range(0, R, 128):
        ri = min(128, R - i0)
        for j0 in range(0, Cc, 128):
            rj = min(128, Cc - j0)
            ps = pp.tile([128, 128], F32, tag="mps")
            nc.tensor.transpose(ps[0:rj, 0:ri], src_ap[i0:i0 + ri, j0:j0 + rj], ident[0:ri, 0:ri])
            nc.scalar.copy(out=dst_ap[j0:j0 + rj, i0:i0 + ri], in_=ps[0:rj, 0:ri])


def _build_body(nc, tc, x_d, w_d, bn_d, out_d):
    from contextlib import ExitStack
    es = ExitStack()
    pool = es.enter_context(tc.tile_pool(name="persist", bufs=1))
    wk = es.enter_context(tc.tile_pool(name="work", bufs=2))
    pps = es.enter_context(tc.tile_pool(name="psum_s", bufs=3, space="PSUM"))
    ppm = es.enter_context(tc.tile_pool(name="psum_m", bufs=2, space="PSUM"))
    _build_inner(nc, tc, x_d, w_d, bn_d, out_d, pool, wk, pps, ppm)
    es.close()


def _build_inner(nc, tc, x_d, w_d, bn_d, out_d, pool, wk, pps, ppm):

    ident = pool.tile([128, 128], F32)
    make_identity(nc, ident)

    # ---- persistent x storage ----
    # X1 = [x1(64); ones], X2 = [x2(64); ones], X3 = [x3(128); ones]
    X1 = pool.tile([65, N], F32)
    X2 = pool.tile([65, N], F32)
    X3 = pool.tile([128, N], F32)
    X4a = pool.tile([128, N], F32)
    X4b = pool.tile([128, N], F32)
    nc.vector.memset(X1[64:65, :], 1.0)
    nc.vector.memset(X2[64:65, :], 1.0)

    ones128 = pool.tile([1, 128], F32)
    nc.vector.memset(ones128, 1.0)
    onescol = pool.tile([128, 1], F32)
    nc.vector.memset(onescol, 1.0)

    # iota base row for chunk-local -> global index conversion (per 256-chunk)
    baserow = pool.tile([128, 128], F32)
    nc.gpsimd.iota(baserow, pattern=[[256, 16], [0, 8]], base=0,
                   channel_multiplier=0, allow_small_or_imprecise_dtypes=True)
    negone = pool.tile([128, 128], F32)
    nc.vector.memset(negone, -1.0)

    # ---- block 1 input: A1 = [x; 1], B1 = [x; -x^2/2] ----
    A1 = pool.tile([2, N], F32, tag="r1")
    B1 = pool.tile([2, N], F32, tag="r2")
    nc.vector.memset(A1, 1.0)               # row 1 stays ones
    nc.sync.dma_start(out=A1[0:1, :], in_=x_d)
    nc.sync.dma_start(out=B1[0:1, :], in_=x_d)
    for ch in range(8):
        cs = slice(ch * 512, (ch + 1) * 512)
        sq1 = wk.tile([1, 512], F32, tag="sq1")
        nc.vector.tensor_tensor(out=sq1, in0=B1[0:1, cs], in1=B1[0:1, cs], op=AOT.mult)
        nc.vector.tensor_scalar_mul(sq1, sq1, -0.5)
        nc.sync.dma_start(out=B1[1:2, cs], in_=sq1)

    blocks = [
        dict(tag="1", C=1, O=64, w=w_d["w1"]),
        dict(tag="2", C=64, O=64, w=w_d["w2"]),
        dict(tag="3", C=64, O=128, w=w_d["w3"]),
        dict(tag="4", C=128, O=256, w=w_d["w4"]),
    ]
    # per-block A/B score operand tiles (blocks 2,3); block 4 uses X3 + mxx row
    AB = {"1": (A1, B1), "2": (X1, None), "3": (X2, None), "4": (X3, None)}
    Xout = {"1": [X1], "2": [X2], "3": [X3], "4": [X4a, X4b]}

    mxx = pool.tile([1, N], F32, tag="r2")   # -xx/2 row (blocks 2-4)
    Bx = pool.tile([65, N], F32, tag="r1")  # B = [x; -xx/2] for blocks 2,3
    xsqc = pool.tile([128, 512], F32)     # x^2 chunk scratch

    u0 = pool.tile([128, N], F32, tag="u0")
    u1 = pool.tile([128, N], F32, tag="u1")
    u_sb = [u0, u1]

    for blk in blocks:
        tag, C, O = blk["tag"], blk["C"], blk["O"]
        first = tag == "1"
        last = tag == "4"
        # ---------- weight transposes ----------
        # w [O, 2C] -> WnT [C, O] (first C cols), WvT [C, O] (Wc - Wn)^T
        if first:
            WnT = wk.tile([1, 64], F32, tag="wnt")
            WvT = wk.tile([1, 64], F32, tag="wvt")
            nc.sync.dma_start(out=WnT, in_=blk["w"].rearrange("o c -> c o")[0:1, :])
            nc.sync.dma_start(out=WvT, in_=blk["w"].rearrange("o c -> c o")[1:2, :])
            nc.vector.tensor_tensor(out=WvT, in0=WvT, in1=WnT, op=AOT.subtract)
        else:
            WnT = wk.tile([C, O], F32, tag="wnt")
            WvT = wk.tile([C, O], F32, tag="wvt")
            for o0 in range(0, O, 128):
                ow0 = min(128, O - o0)
                wfull = wk.tile([128, 2 * C], F32, tag="wload")
                nc.sync.dma_start(out=wfull[0:ow0, :], in_=blk["w"][o0:o0 + ow0, :])
                _transpose_to(nc, tc, ppm, wk, wfull[0:ow0, 0:C], WnT[:, o0:o0 + ow0], ident)
                _transpose_to(nc, tc, ppm, wk, wfull[0:ow0, C:2 * C], WvT[:, o0:o0 + ow0], ident)
            nc.vector.tensor_tensor(out=WvT, in0=WvT, in1=WnT, op=AOT.subtract)
        sc_bn, bi_bn = _bn_consts(nc, pool, bn_d, tag, O)

        # ---------- score operand setup ----------
        if first:
            A, B = A1, B1
            Ck = 2   # contraction for single-matmul path
            split = False
        else:
            Xin = AB[tag][0]          # [C(+1), N] with ones row at C
            # xx row: per chunk square + PE ones-reduce
            for ch in range(8):
                cs = slice(ch * 512, (ch + 1) * 512)
                nc.vector.tensor_tensor(out=xsqc[0:C, :], in0=Xin[0:C, cs],
                                        in1=Xin[0:C, cs], op=AOT.mult)
                psx = ppm.tile([1, 512], F32, tag="mps")
                nc.tensor.matmul(psx, lhsT=onescol[0:C, :], rhs=xsqc[0:C, :],
                                 start=True, stop=True)
                nc.scalar.mul(out=mxx[:, cs], in_=psx, mul=-0.5)
            if tag in ("2", "3"):
                # B = [x; -xx/2] single tile (copy x rows + mxx row)
                nc.vector.tensor_copy(out=Bx[0:C, :], in_=Xin[0:C, :])
                nc.sync.dma_start(out=Bx[C:C + 1, :], in_=mxx)
                A, B = Xin, Bx
                Ck = C + 1
                split = False
            else:
                A, B = Xin, None
                Ck = C
                split = True

        # ---------- u = Wn @ x, into SBUF for gather ----------
        xin_rows = A[0:C, :] if not first else A1[0:1, :]
        for oh in range(0, O, 128):
            ow = min(128, O - oh)
            for ch in range(8):
                cs = slice(ch * 512, (ch + 1) * 512)
                psu = ppm.tile([128, 512], F32, tag="mps")
                nc.tensor.matmul(psu[0:ow, :], lhsT=WnT[:, oh:oh + ow],
                                 rhs=xin_rows[:, cs], start=True, stop=True)
                nc.scalar.copy(out=u_sb[oh // 128][0:ow, cs], in_=psu[0:ow, :])

        # ---------- per row-tile: scores, topk, gather, postproc ----------
        for mb in range(NTILES // 8):
          idxbat = wk.tile([128, 160], I16, tag="idxbat", bufs=1)
          for m in range(mb * 8, mb * 8 + 8):
            ms = slice(m * 128, (m + 1) * 128)
            V = wk.tile([128, 128], F32, tag="V")
            Iglob = wk.tile([128, 128], F32, tag="Iglob")
            Iloc = wk.tile([128, 128], U16, tag="Iloc")
            for ch in range(8):
                cs = slice(ch * 512, (ch + 1) * 512)
                ps = pps.tile([128, 512], F32, tag="sps")
                if not split:
                    nc.tensor.matmul(ps, lhsT=A[0:Ck, ms], rhs=B[0:Ck, cs],
                                     start=True, stop=True)
                else:
                    nc.tensor.matmul(ps, lhsT=A[0:Ck, ms], rhs=A[0:Ck, cs],
                                     start=True, stop=False)
                    nc.tensor.matmul(ps, lhsT=ones128, rhs=mxx[:, cs],
                                     start=False, stop=True)
                for h in range(2):
                    c = ch * 2 + h
                    vs = slice(c * 8, (c + 1) * 8)
                    hw = slice(h * 256, (h + 1) * 256)
                    nc.vector.max(out=V[:, vs], in_=ps[:, hw])
                    nc.vector.max_index(out=Iloc[:, vs], in_max=V[:, vs],
                                        in_values=ps[:, hw])
            nc.vector.tensor_copy(out=Iglob, in_=Iloc)
            nc.vector.tensor_tensor(out=Iglob, in0=Iglob, in1=baserow, op=AOT.add)

            # merged rounds with kill-diff decode
            idxf = wk.tile([128, 24], F32, tag="idxf")
            vw1 = wk.tile([128, 128], F32, tag="vw1")
            vw2 = wk.tile([128, 128], F32, tag="vw2")
            vw3 = wk.tile([128, 128], F32, tag="vw3")
            Vw = [V, vw1, vw2, vw3]
            killm = wk.tile([128, 128], mybir.dt.uint8, tag="killm")
            for r in range(3):
                m8 = wk.tile([128, 8], F32, tag=f"m8{r}")
                nc.vector.max(out=m8, in_=Vw[r])
                if r == 2:
                    mod = wk.tile([128, 8], F32, tag="mod")
                    nc.vector.memset(mod, NEG)
                    nc.vector.tensor_copy(out=mod[:, 0:4], in_=m8[:, 0:4])
                    rep = mod
                else:
                    rep = m8
                nc.vector.match_replace(out=Vw[r + 1], in_to_replace=rep,
                                        in_values=Vw[r], imm_value=NEG)
                nc.vector.tensor_tensor(out=killm, in0=Vw[r], in1=Vw[r + 1],
                                        op=AOT.not_equal)
                masked = wk.tile([128, 128], F32, tag="masked", bufs=1)
                nc.vector.select(out=masked, mask=killm, on_true=Iglob, on_false=negone)
                nc.vector.max(out=idxf[:, r * 8:(r + 1) * 8], in_=masked)

            ml = m - mb * 8
            nc.vector.tensor_copy(out=idxbat[:, ml * K:(ml + 1) * K], in_=idxf[:, 0:K])
          # batched rewrap: wrapbat[q, ml*160 + t*20 + k] = idxbat[16t+q, ml*20+k]
          wrapbat = wk.tile([128, 8 * 8 * K], I16, tag="wrapbat", bufs=2)
          for t in range(8):
              nc.sync.dma_start(
                  out=wrapbat[0:16, :].rearrange("q (ml tk) -> q ml tk", ml=8)[:, :, t * K:(t + 1) * K],
                  in_=idxbat[t * 16:(t + 1) * 16, :].rearrange("q (ml k) -> q ml k", ml=8))
          for cg in range(1, 8):
              nc.sync.dma_start(out=wrapbat[cg * 16:(cg + 1) * 16, :], in_=wrapbat[0:16, :])
          for m in range(mb * 8, mb * 8 + 8):
            ms = slice(m * 128, (m + 1) * 128)
            ml = m - mb * 8
            wrap = wrapbat[:, ml * 8 * K:(ml + 1) * 8 * K]
            # gather u + reduce over k; + v chunk; + bn+lrelu -> X storage
            for oh in range(0, O, 128):
                ow = min(128, O - oh)
                owg = ((ow + 15) // 16) * 16
                gath = wk.tile([128, 128 * K], F32, tag=f"gath{oh // 128}", bufs=1)
                NSPLIT = 4  # ap_gather cost is super-linear in num_idxs; split calls
                npc = (128 * K) // NSPLIT
                for sp in range(NSPLIT):
                    nc.gpsimd.ap_gather(out_ap=gath[0:owg, sp * npc:(sp + 1) * npc],
                                        in_ap=u_sb[oh // 128][0:owg, :],
                                        idxs_ap=wrap[0:owg, sp * (npc // 16):(sp + 1) * (npc // 16)],
                                        channels=owg, num_elems=N, d=1, num_idxs=npc)
                gv = gath[0:ow, :].rearrange("o (t k q) -> o t q k", t=8, k=K, q=16)
                M = wk.tile([128, 128], F32, tag=f"M{oh // 128}")
                nc.vector.tensor_reduce(out=M[0:ow, :].rearrange("o (t q) -> o t q", t=8),
                                        in_=gv, axis=mybir.AxisListType.X, op=AOT.max)
                psv = ppm.tile([128, 128], F32, tag="mps")
                nc.tensor.matmul(psv[0:ow, :], lhsT=WvT[:, oh:oh + ow],
                                 rhs=xin_rows[:, ms], start=True, stop=True)
                nc.vector.tensor_tensor(out=M[0:ow, :], in0=M[0:ow, :],
                                        in1=psv[0:ow, :], op=AOT.add)
                # bn + lrelu
                dsts = Xout[tag]
                dst = dsts[oh // 128] if len(dsts) > 1 else dsts[0]
                h = oh // 128
                nc.scalar.activation(out=dst[0:ow, ms], in_=M[0:ow, :],
                                     func=ACT_F.Prelu,
                                     bias=bi_bn[0:ow, h:h + 1],
                                     scale=sc_bn[0:ow, h:h + 1], alpha=0.2)

    # ---------- final MLP ----------
    # w5 [1024, 512] -> w5T parts per x-source (partition base 0 each)
    w5big = pool.tile([128, 5 * 1024], F32, tag="u0")
    w5T_x1 = w5big[:, 0:1024]
    w5T_x2 = w5big[:, 1024:2048]
    w5T_x3 = w5big[:, 2048:3072]
    w5T_x4a = w5big[:, 3072:4096]
    w5T_x4b = w5big[:, 4096:5120]
    w5l = wk.tile([128, 512], F32, tag="w5load")
    for i in range(8):   # 8 row-tiles of w5 (out-channel tiles)
        nc.sync.dma_start(out=w5l, in_=w_d["w5"][i * 128:(i + 1) * 128, :])
        for j in range(4):
            ps = ppm.tile([128, 128], F32, tag="mps")
            nc.tensor.transpose(ps, w5l[:, j * 128:(j + 1) * 128], ident)
            ic = slice(i * 128, (i + 1) * 128)
            if j == 0:
                nc.scalar.copy(out=w5T_x1[0:64, ic], in_=ps[0:64, :])
                w5s = wk.tile([128, 128], F32, tag="w5shift")
                nc.scalar.copy(out=w5s[64:128, :], in_=ps[64:128, :])
                nc.sync.dma_start(out=w5T_x2[0:64, ic], in_=w5s[64:128, :])
            elif j == 1:
                nc.scalar.copy(out=w5T_x3[:, ic], in_=ps)
            elif j == 2:
                nc.scalar.copy(out=w5T_x4a[:, ic], in_=ps)
            else:
                nc.scalar.copy(out=w5T_x4b[:, ic], in_=ps)
    wregT = pool.tile([128, 8], F32)
    nc.sync.dma_start(out=wregT, in_=w_d["wreg"].rearrange("1 (m p) -> p m", p=128))
    g5 = pool.tile([128, 8], F32)
    b5 = pool.tile([128, 8], F32)
    rm5 = pool.tile([128, 8], F32)
    sc5t = pool.tile([128, 8], F32)
    bi5t = pool.tile([128, 8], F32)
    nc.sync.dma_start(out=g5, in_=bn_d["g5"].rearrange("1 (m p) -> p m", p=128))
    nc.sync.dma_start(out=b5, in_=bn_d["b5"].rearrange("1 (m p) -> p m", p=128))
    nc.sync.dma_start(out=rm5, in_=bn_d["rm5"].rearrange("1 (m p) -> p m", p=128))
    nc.sync.dma_start(out=sc5t, in_=bn_d["rv5"].rearrange("1 (m p) -> p m", p=128))
    nc.vector.tensor_scalar_add(sc5t, sc5t, EPS)
    nc.scalar.sqrt(sc5t, sc5t)
    nc.vector.reciprocal(sc5t, sc5t)
    nc.vector.tensor_tensor(out=sc5t, in0=sc5t, in1=g5, op=AOT.mult)
    nc.vector.tensor_tensor(out=bi5t, in0=rm5, in1=sc5t, op=AOT.mult)
    nc.vector.tensor_tensor(out=bi5t, in0=b5, in1=bi5t, op=AOT.subtract)

    kparts = [(w5T_x1[0:64, :], X1, 64), (w5T_x2[0:64, :], X2, 64), (w5T_x3, X3, 128),
              (w5T_x4a, X4a, 128), (w5T_x4b, X4b, 128)]
    for ch in range(8):
        cs = slice(ch * 512, (ch + 1) * 512)
        psr = pps.tile([1, 512], F32, tag="regps", bufs=1)
        for mt in range(8):
            mslice = slice(mt * 128, (mt + 1) * 128)
            psz = pps.tile([128, 512], F32, tag="sps")
            for kp, (wt, xt, kk) in enumerate(kparts):
                nc.tensor.matmul(psz, lhsT=wt[0:kk, mslice], rhs=xt[0:kk, cs],
                                 start=(kp == 0), stop=(kp == len(kparts) - 1))
            zsb = wk.tile([128, 512], F32, tag="zsb")
            nc.scalar.activation(out=zsb, in_=psz, func=ACT_F.Prelu,
                                 bias=bi5t[:, mt:mt + 1], scale=sc5t[:, mt:mt + 1],
                                 alpha=0.2)
            nc.tensor.matmul(psr, lhsT=wregT[:, mt:mt + 1], rhs=zsb,
                             start=(mt == 0), stop=(mt == 7))
        osb = wk.tile([1, 512], F32, tag="osb", bufs=1)
        nc.scalar.copy(out=osb, in_=psr)
        nc.sync.dma_start(out=out_d[:, cs], in_=osb)


# ----------------------------------------------------------------------------
# Harness entry point: full inputs in, full output out. Shards batch over 8
# NeuronCores (pure data parallel over B), replicates the small weights.
# ----------------------------------------------------------------------------
_NC_CACHE = {}


def _get_nc(num_devices):
    if num_devices not in _NC_CACHE:
        _NC_CACHE[num_devices] = build(num_devices=num_devices)
    return _NC_CACHE[num_devices]


def kernel(**inputs):
    from concourse.bass_utils import run_bass_kernel_spmd

    x = np.asarray(inputs["x"], dtype=np.float32)      # (8, 1, 4096)
    Bn = x.shape[0]
    nc = _get_nc(Bn)
    in_maps = []
    for b in range(Bn):
        m = {"x": x[b]}
        for k in ("w1", "w2", "w3", "w4", "w5", "wreg"):
            m[k] = np.asarray(inputs[k], dtype=np.float32)
        for t in "12345":
            for p in ("g", "b", "rm", "rv"):
                m[p + t] = np.asarray(inputs[p + t], dtype=np.float32)[None, :]
        in_maps.append(m)
    res = run_bass_kernel_spmd(nc, in_maps, core_ids=list(range(Bn)))
    out = np.stack([res.results[b]["out"] for b in range(Bn)], axis=0)
    return out.astype(np.float32)

